# revision 6
# baseline (speedup 1.0000x reference)
"""ChannelTimeAttention Trainium2 kernel.

Reference computation (per (b, c) pair, all independent):
    pooled = AdaptiveAvgPool(x[b, :, c]) -> [t, 8*8]      (7x7 block means)
    q = pooled @ Wq + bq ; k = pooled @ Wk + bk           [t, 32]
    att = softmax(q @ k.T / sqrt(t))                      [t, t]
    out[b, :, c] = att @ x[b, :, c].reshape(t, h*w)

Sharding: data-parallel over b — one batch element per NeuronCore (8 cores).
Each core streams its x slice [t=16, c=64, h=56, w=56] through SBUF once in
8 "packs" of 8 channels, with partition layout (c_local*16 + t).  Per pack:
  DVE two-stage strided reduce  -> pooled sums [128, 64]
  PE  transpose + 2 matmuls     -> q^T, k^T [32, 128]
  PE  full 128x128 cross-score matmul + additive block-diag mask + softmax
  PE  transpose(att) -> block-diagonal lhsT, then att @ v in 7 N=448 chunks
  DMA out.
1/49 (pool mean), 1/sqrt(16) (score scale) are folded into Wq/bq/Wk on host.

DMA-ring schedule (the critical design point, target_regime=memory):
HWDGE DMAs execute FIFO per issuing ring, and the 16 SDMA engines
round-robin between rings that have pending work.  Spreading the 8 input
packs across multiple rings (previous version) made ALL packs stream
concurrently, so pack 0 only landed at ~20us and compute+output work
serialized after the input phase (135us total).  Instead:
  - all 8 input pack DMAs go on ONE ring (nc.sync), issued up-front: pack
    p completes sequentially at full stream rate (~340 GB/s for 1.6MB),
    so compute starts at ~10us and trails the input stream pack by pack;
  - all output DMAs go on the OTHER HWDGE ring (nc.scalar, after the tiny
    consts DMA), so in/out streams share HBM ~50/50 in steady state and
    the total time approaches (in+out bytes)/HBM rate ~ 75us.
"""

import numpy as np

B, T, C, H, W = 8, 16, 64, 56, 56
DS = 8
DIN = DS * DS  # 64
DOUT = 32
HW = H * W  # 3136
CG = 8  # channels per pack
NPACK = C // CG  # 8
P = CG * T  # 128 partitions
NCH = 7  # output free-dim chunks per pack
CHN = HW // NCH  # 448
N_CORES = 8
MASK_NEG = -30.0


def _build_nc():
    import concourse.bacc as bacc
    import concourse.tile as tile
    from concourse import mybir
    from contextlib import ExitStack

    f32 = mybir.dt.float32
    f32r = mybir.dt.float32r
    # Bacc (not raw Bass): its compile() runs generate_event_semaphores /
    # move_matmul_waits_to_ldweights, which legalize multi-wait instructions
    # down to the 1-sync-wait-per-instruction TRN2 codegen limit.
    nc = bacc.Bacc(trn_type="TRN2", num_swdge_queues=2)

    x_h = nc.dram_tensor("x", [T, C, H, W], f32, kind="ExternalInput")
    # all small constants packed into ONE [128, 194] array (one DMA with
    # >=512B per-partition rows — six separate tiny DMAs cost ~25us of
    # latency-bound sub-512B descriptors):
    #   cols 0:128   mask, 128:160 wq (rows 0:64), 160:192 wk (rows 0:64),
    #   col 192 bq (rows 0:32), col 193 bk (rows 0:32)
    cn_h = nc.dram_tensor("consts", [P, 194], f32, kind="ExternalInput")
    out_h = nc.dram_tensor("out", [T, C, H, W], f32, kind="ExternalOutput")

    X = mybir.AxisListType.X
    Exp = mybir.ActivationFunctionType.Exp

    with ExitStack() as ctx:
        tc = ctx.enter_context(tile.TileContext(nc))
        singles = ctx.enter_context(tc.tile_pool(name="singles", bufs=1))
        # bufs=NPACK: every v-DMA writes a fresh slot (all 8 issued up-front)
        vpool = ctx.enter_context(tc.tile_pool(name="vpool", bufs=NPACK))
        vrpool = ctx.enter_context(tc.tile_pool(name="vrpool", bufs=3))
        opool = ctx.enter_context(tc.tile_pool(name="opool", bufs=3))
        small = ctx.enter_context(tc.tile_pool(name="small", bufs=2))
        attpool = ctx.enter_context(tc.tile_pool(name="attpool", bufs=3))
        psA = ctx.enter_context(tc.tile_pool(name="psA", bufs=1, space="PSUM"))
        psB = ctx.enter_context(tc.tile_pool(name="psB", bufs=3, space="PSUM"))

        consts = singles.tile([P, 194], f32)
        # consts ride the scalar/ACT HWDGE ring FIRST (outputs come later on
        # the same ring), keeping the sync ring exclusively for input packs.
        nc.scalar.dma_start(out=consts, in_=cn_h[:])
        mask = consts[:, 0:128]
        wq = consts[0:DIN, 128:160]
        wk = consts[0:DIN, 160:192]
        bq = consts[0:DOUT, 192:193]
        bk = consts[0:DOUT, 193:194]
        ident = singles.tile([P, P], f32)

        x_ap = x_h[:]
        out_ap = out_h[:]

        # All 8 input DMAs issued up-front on the SAME HWDGE ring (nc.sync):
        # FIFO execution means pack p completes before pack p+1 starts, each
        # at the full single-stream rate, so the consumer pipeline starts as
        # early as possible.
        v_tiles = []
        for p in range(NPACK):
            c0 = p * CG
            # v[(t*8 + c_l), h*w] = x[t, c0+c_l, h, w]  — t-MAJOR partition
            # order, so the DMA walks DRAM nearly sequentially (100KB runs).
            # (Keep each DMA full-128-partition — 64-partition halves run at
            # half port bandwidth.)
            v = vpool.tile([P, HW], f32, tag="v")
            src = x_ap[:, c0 : c0 + CG, :, :].rearrange("t c h w -> t c (h w)")
            nc.sync.dma_start(out=v[:], in_=src)
            v_tiles.append(v)

        # identity built on-chip (gpsimd memset + affine_select) — no DMA,
        # ready within a few us of kernel start
        from concourse.masks import make_identity

        make_identity(nc, ident[:])

        # Two-stage software pipeline: stage 1 (pool -> q/k -> scores ->
        # softmax -> att^T) for pack p is emitted BEFORE stage 2 (att @ v ->
        # out DMA) of pack p-1, so the next pack's DVE/ACT work is
        # prioritized ahead of the previous pack's PSUM evacuation and the
        # per-pack cross-engine dependency cycle spans two packs instead of
        # one.
        stage2 = []  # (pack_idx, v, attT)

        def emit_stage1(p):
            v = v_tiles[p]
            # round v to fp32r for the PE (ACT) — the BIR verifier requires
            # fp32r matmul operands to come from a rounding instruction, so
            # a plain bitcast of the DMA-written f32 tile is rejected.
            v_mm = vrpool.tile([P, HW], f32r, tag="vr")
            nc.scalar.copy(out=v_mm, in_=v)

            # ---- adaptive avg pool (sum; /49 folded into weights) ----
            tmp = small.tile([P, H, DS], f32, tag="tmp")
            nc.vector.reduce_sum(
                out=tmp[:],
                in_=v[:].rearrange("p (h j vv) -> p h j vv", h=H, j=DS, vv=7),
                axis=X,
            )
            pooled = small.tile([P, DS, DS], f32, tag="pooled")
            nc.vector.reduce_sum(
                out=pooled[:],
                in_=tmp[:].rearrange("p (i u) j -> p i j u", i=DS, u=7),
                axis=X,
            )

            # ---- pooled^T via PE so q/k matmuls contract over d_in ----
            pooledT_ps = psA.tile([DIN, P], f32, tag="pooledT_ps")
            nc.tensor.transpose(
                pooledT_ps, pooled[:].rearrange("p i j -> p (i j)"), ident
            )
            pooledT = small.tile([DIN, P], f32, tag="pooledT")
            nc.scalar.copy(pooledT, pooledT_ps)

            # ---- q^T, k^T [32, 128] ----
            qT_ps = psA.tile([DOUT, P], f32, tag="qT_ps")
            nc.tensor.matmul(qT_ps, lhsT=wq, rhs=pooledT, start=True, stop=True)
            kT_ps = psA.tile([DOUT, P], f32, tag="kT_ps")
            nc.tensor.matmul(kT_ps, lhsT=wk, rhs=pooledT, start=True, stop=True)
            qT = small.tile([DOUT, P], f32, tag="qT")
            nc.vector.tensor_scalar_add(out=qT, in0=qT_ps, scalar1=bq)
            kT = small.tile([DOUT, P], f32, tag="kT")
            nc.vector.tensor_scalar_add(out=kT, in0=kT_ps, scalar1=bk)

            # ---- full cross scores [128, 128]; only diag blocks survive mask
            sc_ps = psA.tile([P, P], f32, tag="sc_ps")
            nc.tensor.matmul(sc_ps, lhsT=qT, rhs=kT, start=True, stop=True)
            scm = small.tile([P, P], f32, tag="scm")
            nc.vector.tensor_add(out=scm, in0=sc_ps, in1=mask)

            # ---- softmax along free dim ----
            negm = small.tile([P, 1], f32, tag="negm")
            nc.vector.reduce_max(out=negm, in_=scm, axis=X, negate=True)
            e = small.tile([P, P], f32, tag="e")
            ssum = small.tile([P, 1], f32, tag="ssum")
            nc.scalar.activation(
                out=e, in_=scm, func=Exp, bias=negm, scale=1.0, accum_out=ssum
            )
            rinv = small.tile([P, 1], f32, tag="rinv")
            nc.vector.reciprocal(rinv, ssum)
            att = small.tile([P, P], f32, tag="att")
            nc.vector.tensor_scalar_mul(out=att, in0=e, scalar1=rinv)

            # ---- att^T (block-diagonal) becomes the stationary operand ----
            attT_ps = psA.tile([P, P], f32, tag="attT_ps")
            nc.tensor.transpose(attT_ps, att, ident)
            attT = attpool.tile([P, P], f32r, tag="attT")
            nc.scalar.copy(attT, attT_ps)
            stage2.append((p, v_mm, attT))

        def emit_stage2(p, v_mm, attT):
            c0 = p * CG
            o = opool.tile([P, HW], f32, tag="o")
            # claim the o slot with a cheap DVE op: it absorbs the WAR wait
            # on the out-DMA that previously read this slot
            nc.vector.memset(o[:, 0:1], 0.0)
            for ch in range(NCH):
                sl = slice(ch * CHN, (ch + 1) * CHN)
                ops = psB.tile([P, CHN], f32, tag="ochunk")
                nc.tensor.matmul(
                    ops, lhsT=attT[:], rhs=v_mm[:, sl], start=True, stop=True
                )
                # split PSUM->SBUF evacuation between DVE and ACT (ACT also
                # carries the fp32r round-copy, DVE the pool reduces; 2/5
                # split roughly balances the two)
                if ch in (0, 4):
                    nc.vector.tensor_copy(out=o[:, sl], in_=ops)
                else:
                    nc.scalar.copy(out=o[:, sl], in_=ops)

            # outs all ride the scalar/ACT HWDGE ring, FIFO in pack order;
            # t-major order writes DRAM nearly sequentially as well
            dst = out_ap[:, c0 : c0 + CG, :, :].rearrange("t c h w -> t c (h w)")
            nc.scalar.dma_start(out=dst, in_=o[:])

        for p in range(NPACK):
            emit_stage1(p)
            if p >= 1:
                emit_stage2(*stage2[p - 1])
        emit_stage2(*stage2[NPACK - 1])

    nc.compile()
    return nc


def _host_consts(Wq, bq, Wk, bk):
    # fold pool-mean 1/49 into both weight mats; fold score 1/sqrt(t)=1/4
    # into the q side (weights AND bias)
    wq_eff = (Wq / (49.0 * 4.0)).astype(np.float32)
    bq_eff = (bq / 4.0).astype(np.float32)
    wk_eff = (Wk / 49.0).astype(np.float32)
    bk_eff = bk.astype(np.float32)
    # t-major partition order: row i = (t=i//8, c=i%8); attention pairs
    # (i, j) belong to the same channel iff i%8 == j%8
    idx = np.arange(P)
    same_c = np.equal.outer(idx % CG, idx % CG)
    mask = np.where(same_c, 0.0, MASK_NEG).astype(np.float32)
    consts = np.zeros((P, 194), dtype=np.float32)
    consts[:, 0:128] = mask
    consts[0:DIN, 128:160] = wq_eff
    consts[0:DIN, 160:192] = wk_eff
    consts[0:DOUT, 192] = bq_eff
    consts[0:DOUT, 193] = bk_eff
    return consts


def kernel(x, Wq, bq, Wk, bk):
    from concourse.bass_utils import run_bass_kernel_spmd

    x = np.ascontiguousarray(x, dtype=np.float32)
    consts = _host_consts(Wq, bq, Wk, bk)

    nc = _build_nc()
    in_maps = [{"x": x[i], "consts": consts} for i in range(N_CORES)]
    res = run_bass_kernel_spmd(nc, in_maps, core_ids=list(range(N_CORES)))
    global LAST_RUN
    LAST_RUN = res
    out = np.stack([r["out"] for r in res.results], axis=0)
    return out


LAST_RUN = None
